# revision 1
# baseline (speedup 1.0000x reference)
"""Trainium2 Bass kernel for nn_Decoder_35837207118002 (retrieval_knn).

Problem: b=1, n_pre=8192, n_cur=16384, K=8.
  cur2pre[j] = argmin_i D[i,j]           (nearest pre for each cur)
  knn_idx[i] = 8 smallest D[i,:] (indices into cur)
  mask[i,k]  = (cur2pre[knn_idx[i,k]] == i)
  out[i]     = sum_k mask*dist / upsample[i],  dist = ||pre_i - cur_knn||

Sharding: pre split across 8 cores (1024 rows each), cur replicated.

Device strategy (per core), built around two empirical facts probed on
this stack: (1) the PE accumulates matmul products sequentially in
contraction-row order in fp32, bit-replicably from numpy; (2) PE cost is
independent of the contraction dim K (<=128) at 1 cycle/row for bf16.

  ND = 2*p.c - |p|^2 - |c|^2  (= -D) is computed as a K=24 bf16 matmul:
  every fp32 input is split into three exact bf16 pieces (h+m+l), and
  the 24 rank-1 terms reproduce full fp32-level precision at 4x the
  fp32 matmul rate.  Each [128, 2048] PSUM tile (two pre-chunks wide) is
  drained by ACT to bf16 SBUF, which feeds both reductions:
    - DVE: a bf16 2x tensor_tensor max tree computes the per-row window
      maxima W (32 cur cols/window) used for top-8 *selection* only;
    - column maxima: ~9/16 chunks use the gpsimd C-axis tensor_reduce
      (partition direction), the rest a DVE bf16 merge across the 8
      pre-chunks; both ship bf16 block maxima.
  All four engines run concurrently (~120us each of ACT/DVE/Pool busy).

Host side: selects >=top-8 windows per row from W (bf16 ranking with
ties included -- provably covers the true top-8 since bf16 rounding is
monotone), recomputes candidate ND values in reference-style fp32 for
the knn membership, reconstructs the *exact* fp32 column max from the
bf16 block maxima by re-evaluating tied blocks with the bit-exact
device replica, and computes the argmin mask as bitwise equality.  The
final reduction (sqrt / mask / upsample) follows the reference
formulas exactly.
"""

import numpy as np
import ml_dtypes

import concourse.bass as bass
import concourse.tile as tile
import concourse.mybir as mybir
import concourse.bass_utils as bass_utils

F32 = mybir.dt.float32
BF16 = mybir.dt.bfloat16
AX = mybir.AxisListType
OP = mybir.AluOpType

N_CORES = 8
P = 128
N_PRE = 8192
N_CUR = 16384
K = 8
PRE_CORE = N_PRE // N_CORES      # 1024
NCH = PRE_CORE // P              # 8 pre chunks of 128 rows
KB = 24                          # bf16^3 augmented contraction rows
WIN = 32                         # cur columns per selection window
NWIN = N_CUR // WIN              # 512 windows per row
CHUNK = 1024                     # cur columns per column-max block
NCHUNK = N_CUR // CHUNK          # 16
SEL_CAP = 16                     # max windows/row the host will expand
# column-max engine split: these chunks go through the gpsimd C-axis
# reduce; the rest are merged across pre-chunks on the DVE (bf16 2x TT).
# Interleaved 2:1 so the 11.4us Pool stage overlaps neighbouring chunks'
# drains instead of serializing the pipeline.
POOL_CHUNKS = (1, 2, 4, 5, 7, 8, 10, 12, 14)
DVE_CHUNKS = tuple(c for c in range(NCHUNK) if c not in POOL_CHUNKS)
# chunks whose W/column reductions read PSUM directly (no ACT drain) --
# used at the pipeline head/tail where the drain would add latency
PSUM_CHUNKS = ()

_COMPILED = {}


def _split_excess_drain_waits(nc, limit=1):
    """This walrus build encodes very few sem-waits per instruction (a
    Drain tops out at ONE).  Hoist excess waits onto preceding
    single-wait NoOps on the same engine."""
    for f in nc.m.functions:
        for bb in f.blocks:
            insts = list(bb.instructions)
            out = []
            changed = False
            for inst in insts:
                si = inst.sync_info
                waits = list(si.on_wait) if si and si.on_wait else []
                if len(waits) > limit:
                    for kk, w in enumerate(waits[:-limit]):
                        out.append(
                            mybir.InstNoOp(
                                name=f"{inst.name}-wsplit{kk}",
                                engine=inst.engine,
                                ins=[],
                                outs=[],
                                sync_info=mybir.SyncInfo(on_wait=[w], on_update=[]),
                            )
                        )
                    si.on_wait = waits[-limit:]
                    inst.sync_info = si
                    changed = True
                out.append(inst)
            if changed:
                bb.instructions = out


def build_kernel():
    nc = bass.Bass("TRN2", target_bir_lowering=False, debug=False,
                   num_devices=N_CORES)

    NPW = NCH * CHUNK // WIN     # 256 windows per chunk (8 pc x 32)
    pre_b = nc.dram_tensor("pre_b", [KB, PRE_CORE], BF16, kind="ExternalInput").ap()
    cur_b = nc.dram_tensor("cur_b", [KB, N_CUR], BF16, kind="ExternalInput").ap()
    oW = nc.dram_tensor("oW", [P, NCHUNK * NPW], BF16, kind="ExternalOutput").ap()
    # bf16 column maxima, partition-collapsed (Pool chunks): [1, 8192] each
    oMp = nc.dram_tensor("oMp", [P, len(POOL_CHUNKS) * NCH * CHUNK // P], BF16,
                         kind="ExternalOutput").ap()
    # bf16 column maxima, pre-chunk-collapsed (DVE chunks): [128, 1024] each
    oMd = nc.dram_tensor("oMd", [P, len(DVE_CHUNKS) * CHUNK], BF16,
                         kind="ExternalOutput").ap()

    with tile.TileContext(nc) as tc:
        with (
            tc.tile_pool(name="const", bufs=1) as const_pool,
            tc.tile_pool(name="sbig", bufs=4) as sbig_pool,
            tc.tile_pool(name="scr", bufs=2) as scr_pool,
            tc.tile_pool(name="mrow", bufs=2) as mrow_pool,
            tc.tile_pool(name="mmps", bufs=2, space="PSUM") as mm_psum,
        ):
            pre_sb = const_pool.tile([KB, PRE_CORE], BF16)
            nc.gpsimd.dma_start(pre_sb[:], pre_b[:])
            cur_sb = const_pool.tile([KB, N_CUR], BF16)
            for sl in range(16):  # sliced so chunk 0 can start early
                w = N_CUR // 16
                nc.sync.dma_start(cur_sb[:, sl * w:(sl + 1) * w],
                                  cur_b[:, sl * w:(sl + 1) * w])

            W_all = const_pool.tile([P, NCHUNK, NPW], BF16)

            pool_pos = {c: i for i, c in enumerate(POOL_CHUNKS)}
            dve_pos = {c: i for i, c in enumerate(DVE_CHUNKS)}

            for ch in range(NCHUNK):
                col0 = ch * CHUNK
                rhs = cur_sb[:, col0:col0 + CHUNK]
                psum_direct = ch in PSUM_CHUNKS
                if psum_direct:
                    Mrun = mrow_pool.tile([P, CHUNK], BF16,
                                          name=f"Mr_{ch}", tag="mrun")
                else:
                    S_big = sbig_pool.tile([P, NCH, CHUNK], BF16,
                                           name=f"S_{ch}", tag="sbig")
                for q in range(NCH // 2):  # pre-chunk pairs
                    pt = mm_psum.tile([P, 2 * CHUNK], F32,
                                      name=f"pt_{ch}_{q}", tag="mm")
                    for e in range(2):
                        pc = 2 * q + e
                        lhsT = pre_sb[:, pc * P:(pc + 1) * P]
                        for u in range(CHUNK // 512):
                            nc.tensor.matmul(
                                pt[:, e * CHUNK + u * 512:
                                   e * CHUNK + (u + 1) * 512],
                                lhsT, rhs[:, u * 512:(u + 1) * 512],
                                start=True, stop=True,
                            )
                    if psum_direct:
                        # tail/head chunks: consume PSUM directly (no ACT
                        # drain): W windows via treduce, column max via STT.
                        # bf16 rounding commutes with max, so the outputs
                        # are bit-identical to the drained path.
                        nc.vector.tensor_reduce(
                            W_all[:, ch, 2 * q * (CHUNK // WIN):
                                  (2 * q + 2) * (CHUNK // WIN)],
                            pt[:].rearrange("p (w c) -> p w c", c=WIN),
                            axis=AX.X, op=OP.max)
                        if q == 0:
                            nc.vector.tensor_copy(Mrun[:], pt[:, 0:CHUNK])
                        else:
                            nc.vector.scalar_tensor_tensor(
                                out=Mrun[:], in0=pt[:, 0:CHUNK], scalar=0.0,
                                in1=Mrun[:], op0=OP.add, op1=OP.max)
                        nc.vector.scalar_tensor_tensor(
                            out=Mrun[:], in0=pt[:, CHUNK:2 * CHUNK], scalar=0.0,
                            in1=Mrun[:], op0=OP.add, op1=OP.max)
                    else:
                        # drain to bf16 SBUF (feeds both reductions)
                        nc.scalar.copy(
                            S_big[:, 2 * q:2 * q + 2, :]
                            .rearrange("p a b -> p (a b)"),
                            pt[:])

                if not psum_direct:
                    # per-row window maxima via a bf16 2x TT max tree;
                    # step 1 is split per pre-chunk pair so it can start
                    # as soon as that pair's drain lands
                    Sv = S_big[:].rearrange("p a (u z) -> p (a u) z", z=WIN)
                    t1 = scr_pool.tile([P, NPW, 16], BF16, name=f"t1_{ch}",
                                       tag="t1")
                    npq = NPW // 4
                    for qq in range(4):
                        nc.vector.tensor_max(
                            t1[:, qq * npq:(qq + 1) * npq, :],
                            Sv[:, qq * npq:(qq + 1) * npq, 0:16],
                            Sv[:, qq * npq:(qq + 1) * npq, 16:32])
                    t2 = scr_pool.tile([P, NPW, 8], BF16, name=f"t2_{ch}",
                                       tag="t2")
                    nc.vector.tensor_max(t2[:], t1[:, :, 0:8], t1[:, :, 8:16])
                    t3 = scr_pool.tile([P, NPW, 4], BF16, name=f"t3_{ch}",
                                       tag="t3")
                    nc.vector.tensor_max(t3[:], t2[:, :, 0:4], t2[:, :, 4:8])
                    t4 = scr_pool.tile([P, NPW, 2], BF16, name=f"t4_{ch}",
                                       tag="t4")
                    nc.vector.tensor_max(t4[:], t3[:, :, 0:2], t3[:, :, 2:4])
                    nc.vector.tensor_max(W_all[:, ch, :], t4[:, :, 0],
                                         t4[:, :, 1])

                if ch in pool_pos:
                    # gpsimd C-axis reduce: per-(pc, col) max over partitions
                    Mrow = mrow_pool.tile([1, NCH * CHUNK], BF16,
                                          name=f"M_{ch}", tag="mrow")
                    nc.gpsimd.tensor_reduce(
                        Mrow[:], S_big[:], axis=AX.C, op=OP.max)
                    w = NCH * CHUNK // P   # 64
                    pos = pool_pos[ch]
                    nc.sync.dma_start(oMp[:, pos * w:(pos + 1) * w], Mrow[:])
                elif not psum_direct:
                    # DVE bf16 pairwise merge tree across the 8 pre-chunks
                    Mrun = mrow_pool.tile([P, CHUNK], BF16,
                                          name=f"Mr_{ch}", tag="mrun")
                    u1 = scr_pool.tile([P, 4, CHUNK], BF16, name=f"u1_{ch}",
                                       tag="u1")
                    for k in range(4):
                        nc.vector.tensor_max(u1[:, k, :], S_big[:, 2 * k, :],
                                             S_big[:, 2 * k + 1, :])
                    u2 = scr_pool.tile([P, 2, CHUNK], BF16, name=f"u2_{ch}",
                                       tag="u2")
                    for k in range(2):
                        nc.vector.tensor_max(u2[:, k, :], u1[:, 2 * k, :],
                                             u1[:, 2 * k + 1, :])
                    nc.vector.tensor_max(Mrun[:], u2[:, 0, :], u2[:, 1, :])
                if ch not in pool_pos:
                    pos = dve_pos[ch]
                    nc.sync.dma_start(oMd[:, pos * CHUNK:(pos + 1) * CHUNK],
                                      Mrun[:])
                # ship this chunk's W windows (avoids one big tail DMA)
                nc.sync.dma_start(oW[:, ch * NPW:(ch + 1) * NPW],
                                  W_all[:, ch, :])

    _split_excess_drain_waits(nc)
    return nc


def _split3(x):
    """fp32 -> three exact bf16 pieces (h, m, l), x == h + m + l + O(2^-24)."""
    x = np.asarray(x, np.float32)
    h = x.astype(ml_dtypes.bfloat16).astype(np.float32)
    r = (x - h).astype(np.float32)
    m = r.astype(ml_dtypes.bfloat16).astype(np.float32)
    l = (r - m).astype(np.float32).astype(ml_dtypes.bfloat16).astype(np.float32)
    return h, m, l


def _build_aug(p, c):
    """24-row bf16 augmented matrices with ND = sum_k A[k,i]*B[k,j].

    Row order puts the main (hh) terms first so the sequential PE
    accumulation follows the reference's 5-term magnitude profile.
    """
    psq = ((p[0] * p[0] + p[1] * p[1]) + p[2] * p[2]).astype(np.float32)
    csq = ((c[0] * c[0] + c[1] * c[1]) + c[2] * c[2]).astype(np.float32)
    ph, pm, pl = _split3(p)
    ch, cm, cl = _split3(2.0 * c)
    qh, qm, ql = _split3(psq)
    sh, sm, sl = _split3(csq)
    A = np.zeros((KB, p.shape[1]), np.float32)
    B = np.zeros((KB, c.shape[1]), np.float32)
    r = 0
    A[r:r+3] = ph; B[r:r+3] = ch; r += 3
    A[r] = qh; B[r] = -1.0; r += 1
    A[r] = 1.0; B[r] = -sh; r += 1
    A[r:r+3] = ph; B[r:r+3] = cm; r += 3
    A[r:r+3] = pm; B[r:r+3] = ch; r += 3
    A[r] = qm; B[r] = -1.0; r += 1
    A[r] = 1.0; B[r] = -sm; r += 1
    A[r:r+3] = ph; B[r:r+3] = cl; r += 3
    A[r:r+3] = pl; B[r:r+3] = ch; r += 3
    A[r:r+3] = pm; B[r:r+3] = cm; r += 3
    A[r] = ql; B[r] = -1.0; r += 1
    A[r] = 1.0; B[r] = -sl; r += 1
    assert r == KB
    return A.astype(ml_dtypes.bfloat16), B.astype(ml_dtypes.bfloat16)


def _nd_device(A, B, ii, jj):
    """Bit-exact replica of the device 24-term sequential fp32 sum for
    index arrays ii (pre), jj (cur)."""
    acc = np.zeros(ii.shape, np.float32)
    for k in range(KB):
        t = (A[k].astype(np.float32)[ii]
             * B[k].astype(np.float32)[jj]).astype(np.float32)
        acc = (acc + t).astype(np.float32)
    return acc


def kernel(pre_xyzs, cur_xyzs, upsample_num, _run_kwargs=None):
    try:
        import jax
        if not any("NC" in str(d) for d in jax.devices()):
            jax.config.update("jax_platforms", "axon")
    except Exception:
        pass
    if "nc" not in _COMPILED:
        _COMPILED["nc"] = build_kernel()
    nc = _COMPILED["nc"]

    p = np.ascontiguousarray(pre_xyzs[0], dtype=np.float32)   # (3, 8192)
    c = np.ascontiguousarray(cur_xyzs[0], dtype=np.float32)   # (3, 16384)
    up = np.ascontiguousarray(upsample_num[0], dtype=np.float32)

    A, B = _build_aug(p, c)
    in_maps = []
    for core in range(N_CORES):
        s = slice(core * PRE_CORE, (core + 1) * PRE_CORE)
        in_maps.append({"pre_b": np.ascontiguousarray(A[:, s]),
                        "cur_b": np.ascontiguousarray(B)})

    try:
        res = bass_utils.run_bass_kernel_spmd(
            nc, in_maps, core_ids=list(range(N_CORES)), **(_run_kwargs or {}))
    except Exception:
        import time
        time.sleep(5)
        res = bass_utils.run_bass_kernel_spmd(
            nc, in_maps, core_ids=list(range(N_CORES)), **(_run_kwargs or {}))
    _COMPILED["last_results"] = res

    # ---- host reduction ----
    # Exact column max per cur point, reconstructed from the device's
    # bf16 block maxima: bf16 rounding is monotone, so the true fp32
    # argmax row lives in a block whose bf16 max attains the global bf16
    # max.  Evaluate those blocks' rows with the bit-exact device
    # replica and take the true max.
    NW64 = NCH * CHUNK // P                            # 64
    Bp = np.empty((len(POOL_CHUNKS), N_CORES, NCH, CHUNK), np.float32)
    Bd = np.empty((len(DVE_CHUNKS), N_CORES, P, CHUNK), np.float32)
    for core in range(N_CORES):
        Mp = res.results[core]["oMp"]                  # (128, 10*64) bf16
        Md = res.results[core]["oMd"]                  # (128, 6*1024) bf16
        for i, ch in enumerate(POOL_CHUNKS):
            Bp[i, core] = (Mp[:, i * NW64:(i + 1) * NW64]
                           .astype(np.float32).reshape(NCH, CHUNK))
        for i, ch in enumerate(DVE_CHUNKS):
            Bd[i, core] = (Md[:, i * CHUNK:(i + 1) * CHUNK]
                           .astype(np.float32))

    m_exact = np.empty(N_CUR, np.float32)
    cols = np.arange(CHUNK)
    for i, ch in enumerate(POOL_CHUNKS):
        Bf = Bp[i].reshape(N_CORES * NCH, CHUNK)       # 64 blocks of 128 rows
        vmax = Bf.max(0)
        first = np.argmax(Bf, axis=0)                  # first tied block
        co, pc = first // NCH, first % NCH
        base = co * PRE_CORE + pc * P                  # (1024,)
        ii = base[:, None] + np.arange(P)[None, :]     # (1024, 128)
        jj = np.broadcast_to((ch * CHUNK + cols)[:, None], ii.shape)
        m1 = _nd_device(A, B, ii, jj).max(1)
        cnt = (Bf == vmax).sum(0)
        for col in np.nonzero(cnt > 1)[0]:
            blks = np.nonzero(Bf[:, col] == vmax[col])[0]
            for b in blks:
                if b == first[col]:
                    continue
                bb = (b // NCH) * PRE_CORE + (b % NCH) * P
                v = _nd_device(A, B, bb + np.arange(P),
                               np.full(P, ch * CHUNK + col)).max()
                m1[col] = max(m1[col], v)
        m_exact[ch * CHUNK:(ch + 1) * CHUNK] = m1
    for i, ch in enumerate(DVE_CHUNKS):
        Bf = Bd[i].reshape(N_CORES * P, CHUNK)         # 1024 blocks of 8 rows
        vmax = Bf.max(0)
        first = np.argmax(Bf, axis=0)
        co, pp = first // P, first % P
        base = co * PRE_CORE + pp                      # (1024,)
        ii = base[:, None] + np.arange(NCH)[None, :] * P
        jj = np.broadcast_to((ch * CHUNK + cols)[:, None], ii.shape)
        m1 = _nd_device(A, B, ii, jj).max(1)
        cnt = (Bf == vmax).sum(0)
        for col in np.nonzero(cnt > 1)[0]:
            blks = np.nonzero(Bf[:, col] == vmax[col])[0]
            for b in blks:
                if b == first[col]:
                    continue
                bb = (b // P) * PRE_CORE + (b % P)
                v = _nd_device(A, B, bb + np.arange(NCH) * P,
                               np.full(NCH, ch * CHUNK + col)).max()
                m1[col] = max(m1[col], v)
        m_exact[ch * CHUNK:(ch + 1) * CHUNK] = m1
    m_global = m_exact

    # reference-style fp32 ingredients for membership selection
    psq = ((p[0] * p[0] + p[1] * p[1]) + p[2] * p[2]).astype(np.float32)
    csq = ((c[0] * c[0] + c[1] * c[1]) + c[2] * c[2]).astype(np.float32)
    cur_pts = np.ascontiguousarray(c.T)                # (16384, 3)

    out = np.empty((1, N_PRE), np.float32)
    for core in range(N_CORES):
        # oW: [p, ch, pc*32 + w]; row (pc, p) windows g = ch*32 + w
        Wd = res.results[core]["oW"].reshape(P, NCHUNK, NCH, WIN)
        Wv = (np.ascontiguousarray(Wd.transpose(2, 0, 1, 3))
              .reshape(PRE_CORE, NWIN))               # [row_in_core, g] bf16
        Wf = Wv.astype(np.float32)
        # ties-included >= 8th-largest window selection, capped at SEL_CAP
        t8 = -np.partition(-Wf, K - 1, axis=1)[:, K - 1:K]
        selmask = Wf >= t8
        # rank by value to cap selection deterministically
        order = np.argsort(-Wf, axis=1, kind="stable")[:, :SEL_CAP]
        sel = np.where(
            np.take_along_axis(selmask, order, axis=1), order, -1)  # (1024, 16)

        rows = np.arange(PRE_CORE)
        gi = core * PRE_CORE + rows                    # global pre indices
        # candidate cur columns: selected windows expanded to 32 cols
        wsel = sel[:, :, None] * WIN + np.arange(WIN)[None, None, :]
        wsel = wsel.reshape(PRE_CORE, -1)              # (1024, 512)
        valid = sel[:, :, None].repeat(WIN, 2).reshape(PRE_CORE, -1) >= 0

        # reference-style candidate values for knn membership
        ii = np.repeat(gi, wsel.shape[1]).reshape(PRE_CORE, -1)
        jj = np.where(valid, wsel, 0)
        dotpc = (p[0][ii] * c[0][jj] + p[1][ii] * c[1][jj]
                 + p[2][ii] * c[2][jj]).astype(np.float32)
        dref = ((psq[ii] + csq[jj]) - 2.0 * dotpc).astype(np.float32)
        dref[~valid] = np.inf
        # top-8 smallest with index tie-break (reference top_k semantics)
        ordc = np.lexsort((jj, dref), axis=1)[:, :K]   # (1024, 8)
        j8 = np.take_along_axis(jj, ordc, axis=1)      # chosen cur indices

        # device-replicated values for the argmin mask
        i8 = np.repeat(gi, K).reshape(PRE_CORE, K)
        nd_dev = _nd_device(A, B, i8, j8)
        mask = (nd_dev == m_global[j8])

        # exact distances (reference formula)
        pre_pts = p.T[gi]                              # (1024, 3)
        diff = (cur_pts[j8] - pre_pts[:, None, :]).astype(np.float32)
        d2 = ((diff[..., 0] * diff[..., 0] + diff[..., 1] * diff[..., 1])
              + diff[..., 2] * diff[..., 2]).astype(np.float32)
        dist = np.sqrt(d2)
        contrib = (dist * mask).sum(-1, dtype=np.float32).astype(np.float32)
        s = slice(core * PRE_CORE, (core + 1) * PRE_CORE)
        out[0, s] = contrib / up[s]
    return out

